# revision 1
# baseline (speedup 1.0000x reference)
"""CrossTuckerLayer kernel for 8x Trainium2 NeuronCores (Bass/Tile).

Computes y = einsum('bnvade,ABCDEF,oA,pB,qC,aD,dE,eF->bnvopq', ...)
reshaped to [b, n, v, o*p, q], data-parallel over the 2048 (b,n,v) samples
(256 per core), with the tiny Tucker factors folded host-side into three
small matrices:

  stage A (PE, fp32):  per sample, x_s viewed as [(a,dh)=128, (dlo,e)=128]
      is the stationary operand; one matmul against W1 [(a,dh), (v,D,E)=8]
      contracts a and d_hi, leaving t[(dlo,e), (v,D,E)] with e on partitions.
  stage B (PE, fp32):  8 accumulating matmuls against the block-diagonal
      G2bd [(dlo,e)=128, 72] (core x a2 folded, d_lo matched to v, with the
      rank-8 output replicated into 32-aligned row groups) -> s2T [72, s].
  stage C (PE, bf16 3-limb): s2 is split on-chip into 3 bf16 limbs
      (h + m + l ~ 24 mantissa bits); W_out = u0 (x) u1 (x) u2 is pre-split
      host-side. The six product terms that matter are packed into one
      K=96 matmul per output chunk (scaled duplicate W groups make every
      row meaningful), so stage C runs at bf16 rate (1 cycle/row) with
      fp32-grade accuracy.
"""

import numpy as np
import ml_dtypes

import concourse.bass as bass
import concourse.bacc as bacc
import concourse.mybir as mybir
from concourse.tile import TileContext
from concourse.bass_utils import run_bass_kernel_spmd

F32 = mybir.dt.float32
BF16 = mybir.dt.bfloat16
BF = ml_dtypes.bfloat16

NCORES = 8
S_TOT = 2048          # 4*64*8 samples
S = S_TOT // NCORES   # 256 per core
FIN = 16 * 16 * 64    # 16384
FOUT = 256 * 128      # 32768
S_BLK = 32            # samples per x DMA block
N_BLK = S // S_BLK    # 8
WIN = 128             # samples per stage-C window (out partition dim)
N_WIN = S // WIN      # 2
BLK_PER_WIN = WIN // S_BLK  # 4
ROWS = 96             # limb/W operand rows (12 groups of 8)
YCHUNK = 512          # psum bank (fp32)
YSTAGE = 4096         # cols per y staging tile / output DMA
NC_PER_YSTAGE = YSTAGE // YCHUNK  # 8
N_YSTAGE = FOUT // YSTAGE         # 8 per window


def _host_weights(core, u0, u1, u2, a0, a1, a2):
    """Fold the Tucker factors into the three on-chip matrices (float64)."""
    a0 = a0.astype(np.float64)
    a1 = a1.astype(np.float64)
    a2 = a2.astype(np.float64)

    # W1 [(a,dh)=128, (v,D,E)=8]
    a1r = a1.reshape(8, 2, 2)  # [dh, v, E]
    W1 = (a0[:, None, None, :, None] * a1r[None, :, :, None, :])  # [a,dh,v,D,E]
    W1 = W1.reshape(128, 8)

    # G2 [de, e, abc] = sum_F a2[e,F] * core[(A,B,C),(D,E,F)]
    core_mat = core.astype(np.float64).reshape(8, 4, 2)  # [abc, de, F]
    G2 = np.einsum("eF,zdF->dez", a2, core_mat)  # [de, e, abc]

    # G2bd [(dlo,e)=128, (k=(v,de), group, abc) = 8*ROWS], nonzero iff
    # dlo == v; s2 is replicated into all 12 groups.
    G2bd6 = np.zeros((2, 64, 2, 4, ROWS // 8, 8))  # [dlo,e,v,de,grp,abc]
    for v in range(2):
        for g in range(ROWS // 8):
            G2bd6[v, :, v, :, g, :] = np.transpose(G2, (1, 0, 2))
    G2bd = G2bd6.reshape(128, 8 * ROWS)

    # W_out [abc=8, opq=32768]
    Wout = np.einsum(
        "oA,pB,qC->ABCopq",
        u0.astype(np.float64), u1.astype(np.float64), u2.astype(np.float64),
    ).reshape(8, FOUT)

    # 3-limb bf16 split of W_out. The limb operand rows are
    # [h x4 | m x4 | l x4]; pairing with scaled W groups makes every row
    # meaningful (scaling by 1/2 and 1/4 is exact in bf16):
    #   h rows: H/2 + M + L + H/2          = hH + hM + hL
    #   m rows: H/2 + M/2 + H/2 + M/2      = mH + mM
    #   l rows: H/4 x4                     = lH
    Wh = Wout.astype(BF)
    Wm = (Wout - Wh.astype(np.float64)).astype(BF)
    Wl = (Wout - Wh.astype(np.float64) - Wm.astype(np.float64)).astype(BF)
    Wh2 = (Wh.astype(np.float64) * 0.5).astype(BF)
    Wm2 = (Wm.astype(np.float64) * 0.5).astype(BF)
    Wh4 = (Wh.astype(np.float64) * 0.25).astype(BF)
    Wstack = np.concatenate(
        [Wh2, Wm, Wl, Wh2, Wh2, Wm2, Wh2, Wm2, Wh4, Wh4, Wh4, Wh4], axis=0
    )  # [96, FOUT]

    return (
        W1.astype(np.float32),
        G2bd.astype(np.float32),
        np.ascontiguousarray(Wstack),
    )


def _build(reps=1):
    nc = bacc.Bacc("TRN2", target_bir_lowering=False, debug=False)
    x_d = nc.dram_tensor("x", [S, FIN], F32, kind="ExternalInput")
    w1_d = nc.dram_tensor("w1", [128, 8], F32, kind="ExternalInput")
    g2_d = nc.dram_tensor("g2", [128, 8 * ROWS], F32, kind="ExternalInput")
    wl_d = nc.dram_tensor("wl", [ROWS, FOUT], BF16, kind="ExternalInput")
    y_d = nc.dram_tensor("y", [S, FOUT], F32, kind="ExternalOutput")

    with TileContext(nc) as tc:
        with (
            tc.tile_pool(name="consts", bufs=1) as cpool,
            tc.tile_pool(name="xp", bufs=4) as xp,
            tc.tile_pool(name="tp", bufs=2) as tp,
            tc.tile_pool(name="s2p", bufs=2) as s2p,
            tc.tile_pool(name="yp", bufs=3) as yp,
            tc.tile_pool(name="psA", bufs=2, space=bass.MemorySpace.PSUM) as psA,
            tc.tile_pool(name="psB", bufs=2, space=bass.MemorySpace.PSUM) as psB,
            tc.tile_pool(name="psC", bufs=3, space=bass.MemorySpace.PSUM) as psC,
        ):
            w1 = cpool.tile([128, 8], F32)
            nc.sync.dma_start(w1[:], w1_d[:])
            g2 = cpool.tile([128, 8 * ROWS], F32)
            nc.sync.dma_start(g2[:], g2_d[:])
            # W-limb stack (x loads ride the sync ring, wl + y stores the
            # scalar ring, so the streams don't serialize behind each other)
            wl = cpool.tile([ROWS, FOUT], BF16)
            nc.scalar.dma_start(wl[:], wl_d[:])

            def emit_block(blk, s2_ps):
                bw = blk % BLK_PER_WIN
                x_t = xp.tile([128, S_BLK * 128], F32, tag="x", name="x_t")
                src = x_d[blk * S_BLK:(blk + 1) * S_BLK, :].rearrange(
                    "s (p f) -> s p f", p=128
                ).transpose([1, 0, 2])
                nc.sync.dma_start(x_t[:], src)

                # stage A: one matmul per sample (x_s stationary)
                t_ps = psA.tile([128, S_BLK * 8], F32, tag="tps", name="t_ps")
                for sl in range(S_BLK):
                    nc.tensor.matmul(
                        t_ps[:, sl * 8:(sl + 1) * 8],
                        x_t[:, sl * 128:(sl + 1) * 128],
                        w1[:],
                        start=True, stop=True,
                    )
                t_sb = tp.tile([128, S_BLK * 8], F32, tag="tsb", name="t_sb")
                nc.vector.tensor_copy(t_sb[:], t_ps[:])

                # stage B: contract (dlo, e); accumulate all 8 (v,de)
                t_v = t_sb.rearrange("p (s k) -> p s k", k=8)
                for k in range(8):
                    nc.tensor.matmul(
                        s2_ps[:, bw * S_BLK:(bw + 1) * S_BLK],
                        g2[:, k * ROWS:(k + 1) * ROWS],
                        t_v[:, :, k],
                        start=(k == 0), stop=(k == 7),
                    )

            def emit_limb(s2_ps):
                # limb rows: h at 0..31, m at 32..63, l at 64..95
                limb = s2p.tile([ROWS, WIN], BF16, tag="limb", name="limb")
                hf = s2p.tile([ROWS, WIN], F32, tag="hf", name="hf")
                r1 = s2p.tile([ROWS, WIN], F32, tag="r1", name="r1")
                r2 = s2p.tile([ROWS, WIN], F32, tag="r2", name="r2")
                nc.vector.tensor_copy(limb[:], s2_ps[:])      # h everywhere
                nc.vector.tensor_copy(hf[:], limb[:])         # upcast h
                nc.vector.tensor_sub(r1[:], s2_ps[:], hf[:])  # r1 = s2 - h
                nc.vector.tensor_copy(limb[32:64, :], r1[32:64, :])   # m
                nc.vector.tensor_copy(limb[64:96, :], r1[64:96, :])   # m @ l rows
                nc.vector.tensor_copy(hf[64:96, :], limb[64:96, :])   # upcast m
                nc.vector.tensor_sub(r2[64:96, :], r1[64:96, :], hf[64:96, :])
                nc.vector.tensor_copy(limb[64:96, :], r2[64:96, :])   # l
                return limb

            def emit_ctile(w, st, limb):
                y_sb = yp.tile([128, YSTAGE], F32, tag="ysb", name="y_sb")
                for c8 in range(NC_PER_YSTAGE):
                    c = st * NC_PER_YSTAGE + c8
                    y_ps = psC.tile([128, YCHUNK], F32, tag="yps", name="y_ps")
                    nc.tensor.matmul(
                        y_ps[:], limb[:],
                        wl[:, c * YCHUNK:(c + 1) * YCHUNK],
                        start=True, stop=True,
                    )
                    dst = y_sb[:, c8 * YCHUNK:(c8 + 1) * YCHUNK]
                    if c8 % 2 == 0:
                        nc.vector.tensor_copy(dst, y_ps[:])
                    else:
                        nc.scalar.copy(dst, y_ps[:])
                nc.scalar.dma_start(
                    y_d[w * WIN:(w + 1) * WIN, st * YSTAGE:(st + 1) * YSTAGE],
                    y_sb[:],
                )

            for _rep in range(reps):
                # window 0 stages A/B
                s2_ps0 = psB.tile([ROWS, WIN], F32, tag="s2ps", name="s2_ps0")
                for bw in range(BLK_PER_WIN):
                    emit_block(bw, s2_ps0)
                limb0 = emit_limb(s2_ps0)
                # window 0 stage C interleaved with window 1 stages A/B
                s2_ps1 = psB.tile([ROWS, WIN], F32, tag="s2ps", name="s2_ps1")
                for st in range(N_YSTAGE):
                    emit_ctile(0, st, limb0)
                    if st < BLK_PER_WIN:
                        emit_block(BLK_PER_WIN + st, s2_ps1)
                limb1 = emit_limb(s2_ps1)
                for st in range(N_YSTAGE):
                    emit_ctile(1, st, limb1)
    nc.compile()
    return nc


_NC_CACHE = []


def _get_nc():
    if not _NC_CACHE:
        _NC_CACHE.append(_build())
    return _NC_CACHE[0]


def run(inputs, trace=False):
    x = np.ascontiguousarray(np.asarray(inputs["x"], dtype=np.float32))
    W1, G2bd, Wstack = _host_weights(
        np.asarray(inputs["core"]),
        np.asarray(inputs["u0"]), np.asarray(inputs["u1"]),
        np.asarray(inputs["u2"]),
        np.asarray(inputs["a0"]), np.asarray(inputs["a1"]),
        np.asarray(inputs["a2"]),
    )
    x_flat = x.reshape(S_TOT, FIN)
    nc = _get_nc()
    in_maps = []
    for i in range(NCORES):
        in_maps.append({
            "x": np.ascontiguousarray(x_flat[i * S:(i + 1) * S]),
            "w1": W1,
            "g2": G2bd,
            "wl": Wstack,
        })
    res = run_bass_kernel_spmd(
        nc, in_maps, core_ids=list(range(NCORES)), trace=trace,
    )
    y = np.concatenate([r["y"] for r in res.results], axis=0)
    y = y.reshape(4, 64, 8, 256, 128)
    return y, res


def kernel(**inputs) -> np.ndarray:
    y, _ = run(inputs, trace=False)
    return y



# revision 2
# speedup vs baseline: 2.0372x; 2.0372x over previous
"""CrossTuckerLayer kernel for 8x Trainium2 NeuronCores (Bass/Tile).

Computes y = einsum('bnvade,ABCDEF,oA,pB,qC,aD,dE,eF->bnvopq', ...)
reshaped to [b, n, v, o*p, q], data-parallel over the 2048 (b,n,v) samples
(256 per core). All HBM I/O is bf16 (harness gate is rel_err < 2e-2; the
bf16 path lands ~2e-3), halving DMA traffic vs fp32.

Host folds the tiny Tucker factors (all <10K params) into two matrices:
  M    [16384, 8] = einsum('ABCDEF,aD,dE,eF->adeABC', core, a0, a1, a2)
  Wout [8, 32768] = einsum('oA,pB,qC->ABCopq', u0, u1, u2)

On-chip, per core (256 samples):
  stage A (PE): s2[8, 256] = sum over 128 fin-chunks of
      M_ck[128f, 8]^T @ x_ck[128f, 256s]; M is the stationary operand so
      each chunk is ONE matmul streaming 256 bf16 columns, and the result
      lands directly in the [8, s] layout stage C needs (no transpose).
  stage C (PE): y[128s, 512] tiles = s2w[8, 128]^T @ Wout[8, 512], 64
      matmuls per 128-sample window; psum -> sbuf bf16 copies alternate
      vector/scalar engines; y stores ride the scalar DMA queue while x
      loads ride the sync queue.
"""

import numpy as np
import ml_dtypes

import concourse.bass as bass
import concourse.bacc as bacc
import concourse.mybir as mybir
from concourse.tile import TileContext
from concourse.bass_utils import run_bass_kernel_spmd

F32 = mybir.dt.float32
BF16 = mybir.dt.bfloat16
BF = ml_dtypes.bfloat16

NCORES = 8
S_TOT = 2048          # 4*64*8 samples
S = S_TOT // NCORES   # 256 per core
FIN = 16 * 16 * 64    # 16384
FOUT = 256 * 128      # 32768
NCK = FIN // 128      # 128 contraction chunks of 128
G_CK = 16             # chunks per x DMA tile
N_G = NCK // G_CK     # 8 x DMAs
WIN = 128             # samples per stage-C window (out partition dim)
N_WIN = S // WIN      # 2
YCHUNK = 512          # psum bank (fp32)
YSTAGE = 4096         # cols per y staging tile / output DMA
NC_PER_YSTAGE = YSTAGE // YCHUNK  # 8
N_YSTAGE = FOUT // YSTAGE         # 8 per window


def _host_weights(core, u0, u1, u2, a0, a1, a2):
    """Fold the Tucker factors into M [128f, 128ck*8] and Wout [8, FOUT]."""
    M = np.einsum(
        "ABCDEF,aD,dE,eF->adeABC",
        core.astype(np.float64), a0.astype(np.float64),
        a1.astype(np.float64), a2.astype(np.float64),
    ).reshape(FIN, 8)
    # SBUF layout [f, ck*8 + r] where fin = ck*128 + f
    Mdev = np.ascontiguousarray(
        M.reshape(NCK, 128, 8).transpose(1, 0, 2).reshape(128, NCK * 8)
    ).astype(BF)

    Wout = np.einsum(
        "oA,pB,qC->ABCopq",
        u0.astype(np.float64), u1.astype(np.float64), u2.astype(np.float64),
    ).reshape(8, FOUT).astype(BF)
    return Mdev, np.ascontiguousarray(Wout)


def _host_x(x):
    """x [2048, FIN] f32 -> per-core dev layout [128f, ck*256 + s] bf16."""
    xb = x.reshape(S_TOT, FIN).astype(BF)
    # [core, s, ck, f] -> [core, f, ck, s]
    xd = np.ascontiguousarray(
        xb.reshape(NCORES, S, NCK, 128).transpose(0, 3, 2, 1)
    ).reshape(NCORES, 128, NCK * S)
    return xd


def _build():
    nc = bacc.Bacc("TRN2", target_bir_lowering=False, debug=False)
    x_d = nc.dram_tensor("x", [128, NCK * S], BF16, kind="ExternalInput")
    m_d = nc.dram_tensor("m", [128, NCK * 8], BF16, kind="ExternalInput")
    wl_d = nc.dram_tensor("wl", [8, FOUT], BF16, kind="ExternalInput")
    y_d = nc.dram_tensor("y", [S, FOUT], BF16, kind="ExternalOutput")

    with TileContext(nc) as tc:
        with (
            tc.tile_pool(name="consts", bufs=1) as cpool,
            tc.tile_pool(name="xp", bufs=N_G) as xp,
            tc.tile_pool(name="s2p", bufs=1) as s2p,
            tc.tile_pool(name="yp", bufs=3) as yp,
            tc.tile_pool(name="psA", bufs=1, space=bass.MemorySpace.PSUM) as psA,
            tc.tile_pool(name="psC", bufs=4, space=bass.MemorySpace.PSUM) as psC,
        ):
            mm = cpool.tile([128, NCK * 8], BF16)
            nc.sync.dma_start(mm[:], m_d[:])
            wl = cpool.tile([8, FOUT], BF16)
            nc.scalar.dma_start(wl[:], wl_d[:])

            # stage A: s2[8, 256] accumulated over 128 chunk matmuls
            sA_ps = psA.tile([8, S], F32)
            x_tiles = []
            for g in range(N_G):
                xg = xp.tile([128, G_CK * S], BF16, tag="xg", name=f"x_{g}")
                nc.sync.dma_start(xg[:], x_d[:, g * G_CK * S:(g + 1) * G_CK * S])
                x_tiles.append(xg)
            for g in range(N_G):
                for ckl in range(G_CK):
                    ck = g * G_CK + ckl
                    nc.tensor.matmul(
                        sA_ps[:],
                        mm[:, ck * 8:(ck + 1) * 8],
                        x_tiles[g][:, ckl * S:(ckl + 1) * S],
                        start=(ck == 0), stop=(ck == NCK - 1),
                    )
            s2bf = s2p.tile([8, S], BF16)
            nc.vector.tensor_copy(s2bf[:], sA_ps[:])

            # stage C: y[w*128+s, opq] = s2bf[:, w]^T @ Wout
            for w in range(N_WIN):
                for st in range(N_YSTAGE):
                    y_sb = yp.tile([128, YSTAGE], BF16, tag="ysb", name="y_sb")
                    for c8 in range(NC_PER_YSTAGE):
                        c = st * NC_PER_YSTAGE + c8
                        y_ps = psC.tile([128, YCHUNK], F32, tag="yps", name="y_ps")
                        nc.tensor.matmul(
                            y_ps[:],
                            s2bf[:, w * WIN:(w + 1) * WIN],
                            wl[:, c * YCHUNK:(c + 1) * YCHUNK],
                            start=True, stop=True,
                        )
                        dst = y_sb[:, c8 * YCHUNK:(c8 + 1) * YCHUNK]
                        if c8 % 2 == 0:
                            nc.vector.tensor_copy(dst, y_ps[:])
                        else:
                            nc.scalar.copy(dst, y_ps[:])
                    nc.scalar.dma_start(
                        y_d[w * WIN:(w + 1) * WIN, st * YSTAGE:(st + 1) * YSTAGE],
                        y_sb[:],
                    )
    nc.compile()
    return nc


_NC_CACHE = []


def _get_nc():
    if not _NC_CACHE:
        _NC_CACHE.append(_build())
    return _NC_CACHE[0]


def run(inputs, trace=False):
    x = np.asarray(inputs["x"], dtype=np.float32)
    Mdev, Wout = _host_weights(
        np.asarray(inputs["core"]),
        np.asarray(inputs["u0"]), np.asarray(inputs["u1"]),
        np.asarray(inputs["u2"]),
        np.asarray(inputs["a0"]), np.asarray(inputs["a1"]),
        np.asarray(inputs["a2"]),
    )
    xd = _host_x(x)
    nc = _get_nc()
    in_maps = []
    for i in range(NCORES):
        in_maps.append({
            "x": xd[i],
            "m": Mdev,
            "wl": Wout,
        })
    res = run_bass_kernel_spmd(
        nc, in_maps, core_ids=list(range(NCORES)), trace=trace,
    )
    y = np.concatenate([np.asarray(r["y"]) for r in res.results], axis=0)
    y = y.astype(np.float32).reshape(4, 64, 8, 256, 128)
    return y, res


def kernel(**inputs) -> np.ndarray:
    y, _ = run(inputs, trace=False)
    return y


# revision 7
# speedup vs baseline: 2.2681x; 1.1133x over previous
"""CrossTuckerLayer kernel for 8x Trainium2 NeuronCores (Bass/Tile).

Computes y = einsum('bnvade,ABCDEF,oA,pB,qC,aD,dE,eF->bnvopq', ...)
reshaped to [b, n, v, o*p, q], data-parallel over the 2048 (b,n,v) samples
(256 per core). All HBM I/O is bf16 (harness gate is rel_err < 2e-2; the
bf16 path lands ~2e-3), halving DMA traffic vs fp32.

Host folds the tiny Tucker factors (all <10K params) into two matrices:
  M    [16384, 8] = einsum('ABCDEF,aD,dE,eF->adeABC', core, a0, a1, a2)
  Wout [8, 32768] = einsum('oA,pB,qC->ABCopq', u0, u1, u2)

On-chip, per core (256 samples):
  stage A (PE): s2[8, 256] = sum over 128 fin-chunks of
      M_ck[128f, 8]^T @ x_ck[128f, 256s]; M is the stationary operand so
      each chunk is ONE matmul streaming 256 bf16 columns, and the result
      lands directly in the [8, s] layout stage C needs (no transpose).
  stage C (PE): y[128s, 512] tiles = s2w[8, 128]^T @ Wout[8, 512], 64
      matmuls per 128-sample window; psum -> sbuf bf16 copies alternate
      vector/scalar engines; y stores ride the scalar DMA queue while x
      loads ride the sync queue.
"""

import numpy as np
import ml_dtypes

import concourse.bass as bass
import concourse.bacc as bacc
import concourse.mybir as mybir
from concourse.tile import TileContext
from concourse.bass_utils import run_bass_kernel_spmd

F32 = mybir.dt.float32
BF16 = mybir.dt.bfloat16
BF = ml_dtypes.bfloat16

NCORES = 8
S_TOT = 2048          # 4*64*8 samples
S = S_TOT // NCORES   # 256 per core
FIN = 16 * 16 * 64    # 16384
FOUT = 256 * 128      # 32768
NCK = FIN // 128      # 128 contraction chunks of 128
G_CK = 16             # chunks per x DMA tile
N_G = NCK // G_CK     # 8 x DMAs
WIN = 128             # samples per stage-C window (out partition dim)
N_WIN = S // WIN      # 2
YCHUNK = 512          # psum bank (fp32)
YSTAGE = 4096         # cols per y staging tile / output DMA
NC_PER_YSTAGE = YSTAGE // YCHUNK  # 8
N_YSTAGE = FOUT // YSTAGE         # 8 per window


def _host_weights(core, u0, u1, u2, a0, a1, a2):
    """Fold the Tucker factors into M [128f, 128ck*8] and Wout [8, FOUT]."""
    M = np.einsum(
        "ABCDEF,aD,dE,eF->adeABC",
        core.astype(np.float64), a0.astype(np.float64),
        a1.astype(np.float64), a2.astype(np.float64),
    ).reshape(FIN, 8)
    # SBUF layout [f, ck*8 + r] where fin = ck*128 + f
    Mdev = np.ascontiguousarray(
        M.reshape(NCK, 128, 8).transpose(1, 0, 2).reshape(128, NCK * 8)
    ).astype(BF)

    Wout = np.einsum(
        "oA,pB,qC->ABCopq",
        u0.astype(np.float64), u1.astype(np.float64), u2.astype(np.float64),
    ).reshape(8, FOUT).astype(BF)
    return Mdev, np.ascontiguousarray(Wout)


def _host_x(x):
    """x [2048, FIN] f32 -> per-core dev layout [128f, ck*256 + s] bf16."""
    xb = x.reshape(S_TOT, FIN).astype(BF)
    # [core, s, ck, f] -> [core, f, ck, s]
    xd = np.ascontiguousarray(
        xb.reshape(NCORES, S, NCK, 128).transpose(0, 3, 2, 1)
    ).reshape(NCORES, 128, NCK * S)
    return xd


def _build():
    nc = bacc.Bacc("TRN2", target_bir_lowering=False, debug=False)
    x_d = nc.dram_tensor("x", [128, NCK * S], BF16, kind="ExternalInput")
    m_d = nc.dram_tensor("m", [128, NCK * 8], BF16, kind="ExternalInput")
    wl_d = nc.dram_tensor("wl", [8, FOUT], BF16, kind="ExternalInput")
    y_d = nc.dram_tensor("y", [S, FOUT], BF16, kind="ExternalOutput")

    with TileContext(nc) as tc:
        with (
            tc.tile_pool(name="consts", bufs=1) as cpool,
            tc.tile_pool(name="xp", bufs=N_G) as xp,
            tc.tile_pool(name="s2p", bufs=1) as s2p,
            tc.tile_pool(name="yp", bufs=4) as yp,
            tc.tile_pool(name="psA", bufs=1, space=bass.MemorySpace.PSUM) as psA,
            tc.tile_pool(name="psC", bufs=6, space=bass.MemorySpace.PSUM) as psC,
        ):
            mm = cpool.tile([128, NCK * 8], BF16)
            nc.sync.dma_start(mm[:], m_d[:])
            wl = cpool.tile([8, FOUT], BF16)
            nc.scalar.dma_start(wl[:], wl_d[:])

            # stage A: s2[8, 256] accumulated over 128 chunk matmuls
            sA_ps = psA.tile([8, S], F32)
            x_tiles = []
            for g in range(N_G):
                xg = xp.tile([128, G_CK * S], BF16, tag="xg", name=f"x_{g}")
                # split the x stream across both HWDGE queues
                eng = nc.sync if g % 2 == 0 else nc.scalar
                eng.dma_start(xg[:], x_d[:, g * G_CK * S:(g + 1) * G_CK * S])
                x_tiles.append(xg)
            for g in range(N_G):
                for ckl in range(G_CK):
                    ck = g * G_CK + ckl
                    nc.tensor.matmul(
                        sA_ps[:],
                        mm[:, ck * 8:(ck + 1) * 8],
                        x_tiles[g][:, ckl * S:(ckl + 1) * S],
                        start=(ck == 0), stop=(ck == NCK - 1),
                    )
            s2bf = s2p.tile([8, S], BF16)
            nc.vector.tensor_copy(s2bf[:], sA_ps[:])

            # stage C: y[w*128+s, opq] = s2bf[:, w]^T @ Wout
            # NOTE: gpsimd/Pool cannot read PSUM on TRN2 (hw fault) — only
            # DVE and Activation take psum->sbuf copies.
            copy_engines = (nc.vector.tensor_copy, nc.scalar.copy)
            ci = 0
            for w in range(N_WIN):
                for st in range(N_YSTAGE):
                    y_sb = yp.tile([128, YSTAGE], BF16, tag="ysb", name="y_sb")
                    for c8 in range(NC_PER_YSTAGE):
                        c = st * NC_PER_YSTAGE + c8
                        y_ps = psC.tile([128, YCHUNK], F32, tag="yps", name="y_ps")
                        nc.tensor.matmul(
                            y_ps[:],
                            s2bf[:, w * WIN:(w + 1) * WIN],
                            wl[:, c * YCHUNK:(c + 1) * YCHUNK],
                            start=True, stop=True,
                        )
                        dst = y_sb[:, c8 * YCHUNK:(c8 + 1) * YCHUNK]
                        copy_engines[ci % 2](dst, y_ps[:])
                        ci += 1
                    # alternate y stores across both HWDGE queues
                    eng = nc.scalar if st % 2 == 0 else nc.sync
                    eng.dma_start(
                        y_d[w * WIN:(w + 1) * WIN, st * YSTAGE:(st + 1) * YSTAGE],
                        y_sb[:],
                    )
    nc.compile()
    return nc


_NC_CACHE = []


def _get_nc():
    if not _NC_CACHE:
        _NC_CACHE.append(_build())
    return _NC_CACHE[0]


def run(inputs, trace=False):
    x = np.asarray(inputs["x"], dtype=np.float32)
    Mdev, Wout = _host_weights(
        np.asarray(inputs["core"]),
        np.asarray(inputs["u0"]), np.asarray(inputs["u1"]),
        np.asarray(inputs["u2"]),
        np.asarray(inputs["a0"]), np.asarray(inputs["a1"]),
        np.asarray(inputs["a2"]),
    )
    xd = _host_x(x)
    nc = _get_nc()
    in_maps = []
    for i in range(NCORES):
        in_maps.append({
            "x": xd[i],
            "m": Mdev,
            "wl": Wout,
        })
    res = run_bass_kernel_spmd(
        nc, in_maps, core_ids=list(range(NCORES)), trace=trace,
    )
    y = np.concatenate([np.asarray(r["y"]) for r in res.results], axis=0)
    y = y.astype(np.float32).reshape(4, 64, 8, 256, 128)
    return y, res


def kernel(**inputs) -> np.ndarray:
    y, _ = run(inputs, trace=False)
    return y


# revision 8
# speedup vs baseline: 2.3076x; 1.0174x over previous
"""CrossTuckerLayer kernel for 8x Trainium2 NeuronCores (Bass/Tile).

Computes y = einsum('bnvade,ABCDEF,oA,pB,qC,aD,dE,eF->bnvopq', ...)
reshaped to [b, n, v, o*p, q], data-parallel over the 2048 (b,n,v) samples
(256 per core). All HBM I/O is bf16 (harness gate is rel_err < 2e-2; this
path lands ~3.4e-3), halving DMA traffic vs fp32.

Host folds the tiny Tucker factors (all <10K params) into two matrices:
  M    [16384, 8] = einsum('ABCDEF,aD,dE,eF->adeABC', core, a0, a1, a2)
  Wout [8, 32768] = einsum('oA,pB,qC->ABCopq', u0, u1, u2)

Per core the 256 samples split into two 128-sample windows so x-in,
stage A, stage C and y-out pipeline:
  stage A (PE): s2_w[8, 128] = sum over 128 fin-chunks of
      M_ck[128f, 8]^T @ x_ck[128f, 128s]; M is the stationary operand so
      the result lands directly in the [8, s] layout stage C needs.
      Window 1's accumulation is interleaved between window 0's stage-C
      matmuls (distinct psum banks, so the open accumulation group is
      hardware-safe; skip_group_check silences the bass-level check).
  stage C (PE): y[128s, 512] tiles = s2_w[8, 128]^T @ Wout[8, 512];
      psum -> sbuf bf16 copies ([128, 1024] spanning 2 banks) alternate
      vector/scalar engines; x loads and y stores alternate between the
      two HWDGE queues (sync + scalar).

DMA issue is staggered via tile-pool reuse (xp bufs=6): the DMA engines
round-robin across all outstanding transfers on a queue, so issuing
everything upfront makes the FIRST tile complete last.
"""

import numpy as np
import ml_dtypes

import concourse.bass as bass
import concourse.bacc as bacc
import concourse.mybir as mybir
from concourse.tile import TileContext
from concourse.bass_utils import run_bass_kernel_spmd

F32 = mybir.dt.float32
BF16 = mybir.dt.bfloat16
BF = ml_dtypes.bfloat16

NCORES = 8
S_TOT = 2048          # 4*64*8 samples
S = S_TOT // NCORES   # 256 per core
FIN = 16 * 16 * 64    # 16384
FOUT = 256 * 128      # 32768
NCK = FIN // 128      # 128 contraction chunks of 128
WIN = 128             # samples per window
N_WIN = S // WIN      # 2
G_CK = 16             # chunks per x DMA tile
N_G = NCK // G_CK     # 8 x tiles per window
YCHUNK = 512          # one matmul's psum cols (fits a 2KB fp32 bank)
YPS = 1024            # psum tile cols (2 banks, 2 matmuls, 1 copy)
YSTAGE = 4096         # cols per y staging tile / output DMA
N_YSTAGE = FOUT // YSTAGE  # 8 per window


def _host_weights(core, u0, u1, u2, a0, a1, a2):
    """Fold the Tucker factors into M [128f, 128ck*8] and Wout [8, FOUT]."""
    M = np.einsum(
        "ABCDEF,aD,dE,eF->adeABC",
        core.astype(np.float64), a0.astype(np.float64),
        a1.astype(np.float64), a2.astype(np.float64),
    ).reshape(FIN, 8)
    # SBUF layout [f, ck*8 + r] where fin = ck*128 + f
    Mdev = np.ascontiguousarray(
        M.reshape(NCK, 128, 8).transpose(1, 0, 2).reshape(128, NCK * 8)
    ).astype(BF)

    Wout = np.einsum(
        "oA,pB,qC->ABCopq",
        u0.astype(np.float64), u1.astype(np.float64), u2.astype(np.float64),
    ).reshape(8, FOUT).astype(BF)
    return Mdev, np.ascontiguousarray(Wout)


def _host_x(x):
    """x [2048, FIN] f32 -> per-core dev layout [128f, w*16K + ck*128 + s]."""
    xb = x.reshape(S_TOT, FIN).astype(BF)
    xd = np.ascontiguousarray(
        xb.reshape(NCORES, N_WIN, WIN, NCK, 128).transpose(0, 4, 1, 3, 2)
    ).reshape(NCORES, 128, N_WIN * FIN)
    return xd


def _build():
    nc = bacc.Bacc("TRN2", target_bir_lowering=False, debug=False)
    x_d = nc.dram_tensor("x", [128, N_WIN * FIN], BF16, kind="ExternalInput")
    m_d = nc.dram_tensor("m", [128, NCK * 8], BF16, kind="ExternalInput")
    wl_d = nc.dram_tensor("wl", [8, FOUT], BF16, kind="ExternalInput")
    y_d = nc.dram_tensor("y", [S, FOUT], BF16, kind="ExternalOutput")

    with TileContext(nc) as tc:
        with (
            tc.tile_pool(name="consts", bufs=1) as cpool,
            tc.tile_pool(name="xp", bufs=6) as xp,
            tc.tile_pool(name="s2p", bufs=2) as s2p,
            tc.tile_pool(name="yp", bufs=6) as yp,
            tc.tile_pool(name="psA", bufs=2, space=bass.MemorySpace.PSUM) as psA,
            tc.tile_pool(name="psC", bufs=3, space=bass.MemorySpace.PSUM) as psC,
        ):
            mm = cpool.tile([128, NCK * 8], BF16)
            nc.sync.dma_start(mm[:], m_d[:])
            wl = cpool.tile([8, FOUT], BF16)
            nc.scalar.dma_start(wl[:], wl_d[:])

            # x DMAs: issue order == consumption order; xp bufs throttle
            # issue so in-flight transfers stay few (round-robin engines
            # otherwise finish everything at once, starving stage A).
            x_tiles = {}
            for w in range(N_WIN):
                for g in range(N_G):
                    i = w * N_G + g
                    xg = xp.tile([128, G_CK * WIN], BF16, tag="xg",
                                 name=f"x_{w}_{g}")
                    eng = nc.sync if i % 2 == 0 else nc.scalar
                    eng.dma_start(
                        xg[:],
                        x_d[:, (w * NCK + g * G_CK) * WIN:
                               (w * NCK + (g + 1) * G_CK) * WIN],
                    )
                    x_tiles[(w, g)] = xg

            sA = [psA.tile([8, WIN], F32, tag="sA", name=f"sA_{w}")
                  for w in range(N_WIN)]
            s2 = [s2p.tile([8, WIN], BF16, tag="s2", name=f"s2_{w}")
                  for w in range(N_WIN)]

            def emit_a_group(w, g):
                for ckl in range(G_CK):
                    ck = g * G_CK + ckl
                    nc.tensor.matmul(
                        sA[w][:],
                        mm[:, ck * 8:(ck + 1) * 8],
                        x_tiles[(w, g)][:, ckl * WIN:(ckl + 1) * WIN],
                        start=(ck == 0), stop=(ck == NCK - 1),
                        skip_group_check=True,
                    )

            ci = 0

            def emit_c_stage(w, st):
                nonlocal ci
                y_sb = yp.tile([128, YSTAGE], BF16, tag="ysb", name="y_sb")
                for h in range(YSTAGE // YPS):
                    y_ps = psC.tile([128, YPS], F32, tag="yps", name="y_ps")
                    for q in range(YPS // YCHUNK):
                        c = st * (YSTAGE // YCHUNK) + h * (YPS // YCHUNK) + q
                        nc.tensor.matmul(
                            y_ps[:, q * YCHUNK:(q + 1) * YCHUNK],
                            s2[w][:],
                            wl[:, c * YCHUNK:(c + 1) * YCHUNK],
                            start=True, stop=True,
                        )
                    dst = y_sb[:, h * YPS:(h + 1) * YPS]
                    if ci % 2 == 0:
                        nc.vector.tensor_copy(dst, y_ps[:])
                    else:
                        nc.scalar.copy(dst, y_ps[:])
                    ci += 1
                eng = nc.scalar if st % 2 == 0 else nc.sync
                eng.dma_start(
                    y_d[w * WIN:(w + 1) * WIN, st * YSTAGE:(st + 1) * YSTAGE],
                    y_sb[:],
                )

            # window 0 stage A
            for g in range(N_G):
                emit_a_group(0, g)
            nc.vector.tensor_copy(s2[0][:], sA[0][:])
            # window 0 stage C, window 1 stage A interleaved
            for st in range(N_YSTAGE):
                emit_c_stage(0, st)
                emit_a_group(1, st)
            nc.vector.tensor_copy(s2[1][:], sA[1][:])
            for st in range(N_YSTAGE):
                emit_c_stage(1, st)
    nc.compile()
    return nc


_NC_CACHE = []


def _get_nc():
    if not _NC_CACHE:
        _NC_CACHE.append(_build())
    return _NC_CACHE[0]


def run(inputs, trace=False):
    x = np.asarray(inputs["x"], dtype=np.float32)
    Mdev, Wout = _host_weights(
        np.asarray(inputs["core"]),
        np.asarray(inputs["u0"]), np.asarray(inputs["u1"]),
        np.asarray(inputs["u2"]),
        np.asarray(inputs["a0"]), np.asarray(inputs["a1"]),
        np.asarray(inputs["a2"]),
    )
    xd = _host_x(x)
    nc = _get_nc()
    in_maps = []
    for i in range(NCORES):
        in_maps.append({
            "x": xd[i],
            "m": Mdev,
            "wl": Wout,
        })
    res = run_bass_kernel_spmd(
        nc, in_maps, core_ids=list(range(NCORES)), trace=trace,
    )
    y = np.concatenate([np.asarray(r["y"]) for r in res.results], axis=0)
    y = y.astype(np.float32).reshape(4, 64, 8, 256, 128)
    return y, res


def kernel(**inputs) -> np.ndarray:
    y, _ = run(inputs, trace=False)
    return y


# revision 9
# speedup vs baseline: 2.5688x; 1.1132x over previous
"""CrossTuckerLayer kernel for 8x Trainium2 NeuronCores (Bass/Tile).

Computes y = einsum('bnvade,ABCDEF,oA,pB,qC,aD,dE,eF->bnvopq', ...)
reshaped to [b, n, v, o*p, q], data-parallel over the 2048 (b,n,v) samples
(256 per core). All HBM I/O is bf16 (harness gate is rel_err < 2e-2; this
path lands ~3.4e-3), halving DMA traffic vs fp32.

Host folds the tiny Tucker factors (all <10K params) into two matrices:
  M    [16384, 8] = einsum('ABCDEF,aD,dE,eF->adeABC', core, a0, a1, a2)
  Wout [8, 32768] = einsum('oA,pB,qC->ABCopq', u0, u1, u2)

Per core the 256 samples split into two 128-sample windows so x-in,
stage A, stage C and y-out pipeline:
  stage A (PE): s2_w[8, 128] = sum over 128 fin-chunks of
      M_ck[128f, 8]^T @ x_ck[128f, 128s]; M is the stationary operand so
      the result lands directly in the [8, s] layout stage C needs.
      Window 1's accumulation is interleaved between window 0's stage-C
      matmuls (distinct psum banks; skip_group_check silences the
      bass-level open-group check, hardware is per-bank).
  s2 is then replicated to partition blocks 0/32/64/96 (one DVE copy +
      three SBUF->SBUF DMAs) so stage C can row-tile the PE.
  stage C (PE): y[128s, 512] tiles = s2_w[8, 128]^T @ Wout[8, 512] with
      K=8 only — so four matmuls run CONCURRENTLY in distinct 32-row
      PE groups via tile_position=(32i, 0) (Wout is staged per row-group
      host-side). psum -> sbuf bf16 copies alternate vector/scalar; x
      loads and y stores alternate between the two HWDGE queues.

DMA issue is staggered via tile-pool reuse (xp bufs=6): the DMA engines
round-robin across all outstanding transfers on a queue, so issuing
everything upfront makes the FIRST tile complete last.
"""

import numpy as np
import ml_dtypes

import concourse.bass as bass
import concourse.bacc as bacc
import concourse.mybir as mybir
from concourse.tile import TileContext
from concourse.bass_utils import run_bass_kernel_spmd

F32 = mybir.dt.float32
BF16 = mybir.dt.bfloat16
BF = ml_dtypes.bfloat16

NCORES = 8
S_TOT = 2048          # 4*64*8 samples
S = S_TOT // NCORES   # 256 per core
FIN = 16 * 16 * 64    # 16384
FOUT = 256 * 128      # 32768
NCK = FIN // 128      # 128 contraction chunks of 128
WIN = 128             # samples per window
N_WIN = S // WIN      # 2
G_CK = 16             # chunks per x DMA tile
N_G = NCK // G_CK     # 8 x tiles per window
YCHUNK = 512          # one matmul's psum cols (fits a 2KB fp32 bank)
YSTAGE = 4096         # cols per y staging tile / output DMA
N_YSTAGE = FOUT // YSTAGE  # 8 per window
NTILE = 4             # concurrent row-group matmuls in stage C
NSLOT = FOUT // YCHUNK // NTILE  # 16 column slots per row-group


def _host_weights(core, u0, u1, u2, a0, a1, a2):
    """Fold the Tucker factors into M [128f, 128ck*8] and the row-group
    staged Wout [128, NSLOT*512]."""
    M = np.einsum(
        "ABCDEF,aD,dE,eF->adeABC",
        core.astype(np.float64), a0.astype(np.float64),
        a1.astype(np.float64), a2.astype(np.float64),
    ).reshape(FIN, 8)
    # SBUF layout [f, ck*8 + r] where fin = ck*128 + f
    Mdev = np.ascontiguousarray(
        M.reshape(NCK, 128, 8).transpose(1, 0, 2).reshape(128, NCK * 8)
    ).astype(BF)

    Wout = np.einsum(
        "oA,pB,qC->ABCopq",
        u0.astype(np.float64), u1.astype(np.float64), u2.astype(np.float64),
    ).reshape(8, FOUT)
    # chunk c of 512 cols -> row-group i = c % 4, col slot j = c // 4
    wl4 = np.zeros((128, NSLOT * YCHUNK), dtype=np.float64)
    for c in range(FOUT // YCHUNK):
        i, j = c % NTILE, c // NTILE
        wl4[32 * i:32 * i + 8, j * YCHUNK:(j + 1) * YCHUNK] = \
            Wout[:, c * YCHUNK:(c + 1) * YCHUNK]
    return Mdev, np.ascontiguousarray(wl4.astype(BF))


def _host_x(x):
    """x [2048, FIN] f32 -> per-core dev layout [128f, w*16K + ck*128 + s]."""
    xb = x.reshape(S_TOT, FIN).astype(BF)
    xd = np.ascontiguousarray(
        xb.reshape(NCORES, N_WIN, WIN, NCK, 128).transpose(0, 4, 1, 3, 2)
    ).reshape(NCORES, 128, N_WIN * FIN)
    return xd


def _build():
    nc = bacc.Bacc("TRN2", target_bir_lowering=False, debug=False)
    x_d = nc.dram_tensor("x", [128, N_WIN * FIN], BF16, kind="ExternalInput")
    m_d = nc.dram_tensor("m", [128, NCK * 8], BF16, kind="ExternalInput")
    wl_d = nc.dram_tensor("wl", [128, NSLOT * YCHUNK], BF16,
                          kind="ExternalInput")
    y_d = nc.dram_tensor("y", [S, FOUT], BF16, kind="ExternalOutput")

    with TileContext(nc) as tc:
        with (
            tc.tile_pool(name="consts", bufs=1) as cpool,
            tc.tile_pool(name="xp", bufs=6) as xp,
            tc.tile_pool(name="s2p", bufs=2) as s2p,
            tc.tile_pool(name="yp", bufs=6) as yp,
            tc.tile_pool(name="psA", bufs=1, space=bass.MemorySpace.PSUM) as psA,
            tc.tile_pool(name="psC", bufs=7, space=bass.MemorySpace.PSUM) as psC,
        ):
            mm = cpool.tile([128, NCK * 8], BF16)
            nc.sync.dma_start(mm[:], m_d[:])
            wl = cpool.tile([128, NSLOT * YCHUNK], BF16)
            nc.scalar.dma_start(wl[:], wl_d[:])

            # x DMAs: issue order == consumption order; xp bufs throttle
            # issue so in-flight transfers stay few (round-robin engines
            # otherwise finish everything at once, starving stage A).
            x_tiles = {}
            for w in range(N_WIN):
                for g in range(N_G):
                    i = w * N_G + g
                    xg = xp.tile([128, G_CK * WIN], BF16, tag="xg",
                                 name=f"x_{w}_{g}")
                    eng = nc.sync if i % 2 == 0 else nc.scalar
                    eng.dma_start(
                        xg[:],
                        x_d[:, (w * NCK + g * G_CK) * WIN:
                               (w * NCK + (g + 1) * G_CK) * WIN],
                    )
                    x_tiles[(w, g)] = xg

            sA = [psA.tile([8, WIN], F32, tag="sA", name=f"sA_{w}")
                  for w in range(N_WIN)]
            s2r = [s2p.tile([128, WIN], BF16, tag="s2", name=f"s2_{w}")
                   for w in range(N_WIN)]

            def emit_a_group(w, g):
                for ckl in range(G_CK):
                    ck = g * G_CK + ckl
                    nc.tensor.matmul(
                        sA[w][:],
                        mm[:, ck * 8:(ck + 1) * 8],
                        x_tiles[(w, g)][:, ckl * WIN:(ckl + 1) * WIN],
                        start=(ck == 0), stop=(ck == NCK - 1),
                        skip_group_check=True,
                    )

            def emit_s2_replicate(w):
                # bf16 downcast into row-group 0, then fan out to 32/64/96
                nc.vector.tensor_copy(s2r[w][0:8, :], sA[w][:])
                for i in range(1, NTILE):
                    nc.sync.dma_start(s2r[w][32 * i:32 * i + 8, :],
                                      s2r[w][0:8, :])

            ci = 0

            def emit_c_stage(w, st):
                nonlocal ci
                y_sb = yp.tile([128, YSTAGE], BF16, tag="ysb", name="y_sb")
                for h in range(2):
                    slot = st * 2 + h
                    pss = []
                    for i in range(NTILE):
                        y_ps = psC.tile([128, YCHUNK], F32, tag="yps",
                                        name="y_ps")
                        nc.tensor.matmul(
                            y_ps[:],
                            s2r[w][32 * i:32 * i + 8, :],
                            wl[32 * i:32 * i + 8,
                               slot * YCHUNK:(slot + 1) * YCHUNK],
                            start=True, stop=True,
                            tile_position=(32 * i, 0),
                        )
                        pss.append(y_ps)
                    for i in range(NTILE):
                        c8 = h * NTILE + i
                        dst = y_sb[:, c8 * YCHUNK:(c8 + 1) * YCHUNK]
                        if ci % 2 == 0:
                            nc.vector.tensor_copy(dst, pss[i][:])
                        else:
                            nc.scalar.copy(dst, pss[i][:])
                        ci += 1
                eng = nc.scalar if st % 2 == 0 else nc.sync
                eng.dma_start(
                    y_d[w * WIN:(w + 1) * WIN, st * YSTAGE:(st + 1) * YSTAGE],
                    y_sb[:],
                )

            # window 0 stage A
            for g in range(N_G):
                emit_a_group(0, g)
            emit_s2_replicate(0)
            # window 0 stage C, window 1 stage A interleaved
            for st in range(N_YSTAGE):
                emit_c_stage(0, st)
                emit_a_group(1, st)
            emit_s2_replicate(1)
            for st in range(N_YSTAGE):
                emit_c_stage(1, st)
    nc.compile()
    return nc


_NC_CACHE = []


def _get_nc():
    if not _NC_CACHE:
        _NC_CACHE.append(_build())
    return _NC_CACHE[0]


def run(inputs, trace=False):
    x = np.asarray(inputs["x"], dtype=np.float32)
    Mdev, wl4 = _host_weights(
        np.asarray(inputs["core"]),
        np.asarray(inputs["u0"]), np.asarray(inputs["u1"]),
        np.asarray(inputs["u2"]),
        np.asarray(inputs["a0"]), np.asarray(inputs["a1"]),
        np.asarray(inputs["a2"]),
    )
    xd = _host_x(x)
    nc = _get_nc()
    in_maps = []
    for i in range(NCORES):
        in_maps.append({
            "x": xd[i],
            "m": Mdev,
            "wl": wl4,
        })
    res = run_bass_kernel_spmd(
        nc, in_maps, core_ids=list(range(NCORES)), trace=trace,
    )
    y = np.concatenate([np.asarray(r["y"]) for r in res.results], axis=0)
    y = y.astype(np.float32).reshape(4, 64, 8, 256, 128)
    return y, res


def kernel(**inputs) -> np.ndarray:
    y, _ = run(inputs, trace=False)
    return y
